# revision 54
# baseline (speedup 1.0000x reference)
"""TRN2 Bass/Tile kernel: 16-head causal multi-head attention.

Problem: x[2,2048,1024], 16 heads x 64, causal softmax attention + out-proj.

Sharding (8 cores): core = b*4 + g  (b = batch 0..1, g = head-group 0..3).
Each core computes heads [4g, 4g+4) for batch b and the partial
out-projection  ctx_g @ Wo[g*256:(g+1)*256, :]  -> [2048, 1024] (bf16).
Host upcasts, sums the 4 partials per batch and adds bo.

On-device layout is fully "transposed" (feature-major):
  xT   [128, 8, 2048]  : xT[p, kt, s]  = x[b, s, kt*128+p]
  QT/KT[128, 2, 2048]  : QT[p, t, s]   = Q^T[t*128+p, s]   (d' = h*64+j on partitions)
  S^T  [128k, 512q]    : per (head, q-chunk, k-tile) block = K @ Q^T
  softmax: no max-subtraction (scores are O(1) by construction: exp is safe);
  denominators via a ones-column appended to V (row 64 of the ctx psum);
  ctx^T [128, 2, 2048] feeds the out-projection directly as lhsT.

Schedule (v16, from v9):
  - Dual-engine exp: each k-tile's two head-pair exps run CONCURRENTLY --
    pr0 as exact table exp on ScalarE, pr1 as a Schraudolph bit-trick exp
    on DVE (one tensor_scalar: bf16 exp bits = round(x*128/ln2 + 16250),
    f32->int16 convert-on-write, tile bitcast int16->bf16; ~3.4% max rel
    err, ~0.014 end-to-end).  This halves the old ScalarE exp pacing.
  - fp8e4m3 DoubleRow Q/K projections: host packs W/x into [p, j, 2, .]
    k-tile pairs; 4 DR matmuls (K=256 each) replace 8 bf16 ones at the
    same per-instruction cadence (2x PE throughput).  Host scales W into
    fp8 range (SCQ/SCK); the eviction's activation descales.  V/Wo stay
    bf16 (fp8 there fails the 2e-2 gate: early causal rows pass V errors
    straight through).
  - PSUM evictions split across both psum-capable engines: o_sb nch0 +
    Q/K bias (Identity+bias AP) + den band on ScalarE; o_sb nch1 + V bias
    add + norm on DVE.
  - Each chunk's norm chain (den evict/bcast/recip/mul) rides as the
    first fillers of the NEXT chunk's attention; ctx(c+1) only needs the
    banks back at step PIPE=3, so chunk boundaries don't stall the PE.
  - Contiguous chunk-0/rest DMA splits for xt/xt8 (strided per-k-tile
    loads ran ~2x slower and stalled the first projection chain); the
    chunk-0 x loads issue from the Activation engine's DMA queue, running
    parallel to the sync-queue weight loads (only safe while ScalarE is
    idle -- mid-kernel issue ops would block its eviction stream).
  - Drip-load balance: chunk 3 (no next-chunk projections) carries both
    trailing out-projection chunks; warmup sized (56) to the parallel
    head-DMA latency.

Schedule (v9):
  - Head pairs (h0,h1)/(h2,h3) write one [128, 2, 512] two-bank S^T psum
    tile; their matmuls row-tile the PE array concurrently (K=64 halves)
    and ONE exp per pair halves ScalarE's per-op fixed cost.
  - ctx: 2 heads col-tiled per psum bank (concurrent on array column
    halves, M=64 each, layout matches ctxT exactly -> no partition-shift
    bounce); denominators as 4-way col-tiled M=1 matmuls into one bank.
  - Normalization: den band -> SBUF (1 op, bf16), bf16 ones-matmul
    broadcasts per pair (col-tiled), reciprocal_approx_fast on 128
    lanes, one DVE mul per pair.  No Log/Exp table switches.
  - The attention phase is exp-paced (ScalarE ~2.2us/k-tile vs ~1.2us of
    PE work); the next chunk's projections and the previous chunk's
    out-projections are split into ~1us pieces on a dedicated aux psum
    bank and drip-fed between k-tiles so the in-order PE queue can chew
    them under the exp stream.
  - Causal mask (affine_select) trimmed to the 128-wide crossing block.
  - Output stored bf16 (halves the 8MB out DMA); host sums in f32.
  - Warmup: full-K N=128 matmuls from memset tiles (no DMA dep) flip the
    PE HAM clock gate to 8/8 through the initial DMA; filler matmuls
    bridge the final norm latency before the tail out-projections.
"""

import os
import sys

for _p in ("/opt/trn_rl_repo",):
    if _p not in sys.path:
        sys.path.insert(0, _p)

import numpy as np

import concourse.bass as bass
import concourse.mybir as mybir
import concourse.tile as tile
from concourse import bacc
from concourse.bass import ts
from concourse.bass_utils import run_bass_kernel_spmd

B, S, D, H, HD = 2, 2048, 1024, 16, 64
GROUPS, HPG, DG = 4, 4, 256  # head groups, heads/group, group width
KT = D // 128  # 8 k-tiles over D
ST = S // 128  # 16 s-tiles
CH = 512  # q-chunk width
QCH = S // CH  # 4 q-chunks
PIPE = int(os.environ.get("BASS_PIPE", "3"))  # ctx trails S^T/exp by this many k-steps
F32 = mybir.dt.float32
I16 = mybir.dt.int16
BF16 = mybir.dt.bfloat16

# Schraudolph exp on DVE: bf16 bits = round(A16*x + (16256 - C16)); bitcast
# int16 -> bf16 gives exp(x) with ~3.4% max rel err (0.0097 end-to-end).
A16 = 128.0 / float(np.log(2.0))
C16 = float(os.environ.get("BASS_C16", "6"))
B16 = 16256.0 - C16
# every SEXP_MOD-th exp op stays on ScalarE (table exp); 0 = all on DVE
SEXP_MOD = int(os.environ.get("BASS_SEXP_MOD", "5"))

_MM_DT_NAME = os.environ.get("BASS_MM_DT", "bf16")
MM_DT = {
    "f32r": mybir.dt.float32r,
    "f32": mybir.dt.float32,
    "bf16": mybir.dt.bfloat16,
}[_MM_DT_NAME]
WARMUP = int(os.environ.get("BASS_WARMUP", "56"))
# fp8e4m3 DoubleRow for the Q/K projections: 2 k-tiles per pass, 2x PE rate.
QK_FP8 = os.environ.get("BASS_QK_FP8", "1") == "1"
FP8 = mybir.dt.float8e4
SCQ = 512.0  # host: Wq*(1/8)*SCQ into fp8; descaled at eviction
SCK = 64.0
# fp8 Q/K SBUF tiles + DoubleRow scores (2 heads per pair at 32-row strips):
# halves the scores stream. Host permutes W columns into head-strip order
# (slot s = dims [32s,32s+32) of each head); the exp descales by 1/(SQ8*SK8).
SC_FP8 = QK_FP8 and os.environ.get("BASS_SC_FP8", "0") == "1"
SQ8 = 8.0  # Q scaled into fp8 range at eviction; exp descales
SK8 = 1.0
QK_SB_DT = FP8 if SC_FP8 else MM_DT
XT_FP8 = os.environ.get("BASS_XT_FP8", "0") == "1"
XT_DT = mybir.dt.float8e4 if XT_FP8 else MM_DT
TAILFILL = int(os.environ.get("BASS_TAILFILL", "12"))
# pair non-crossing k-tiles' denominators: DVE pre-sums e(2m)+e(2m+1),
# one PE den wave per pair instead of two (PE is the wall, DVE has slack)
DEN_PAIR = os.environ.get("BASS_DEN_PAIR", "1") == "1"
WU_N = int(os.environ.get("BASS_WU_N", "128"))


def _np_dt():
    import ml_dtypes

    return ml_dtypes.bfloat16 if _MM_DT_NAME == "bf16" else np.float32


def build_kernel_body(nc, tc, io):
    Exp = mybir.ActivationFunctionType.Exp
    Identity = mybir.ActivationFunctionType.Identity
    exp_ctr = [0]

    consts = tc.alloc_tile_pool(name="consts", bufs=1)
    acts = tc.alloc_tile_pool(name="acts", bufs=1)
    work = tc.alloc_tile_pool(name="work", bufs=2)
    small = tc.alloc_tile_pool(name="small", bufs=2)
    psum = tc.alloc_tile_pool(name="psum", bufs=1, space="PSUM")

    # ---- on-chip constants (no DMA dependency: warmup starts at t=0) ----
    wu_sb = consts.tile([128, 128], MM_DT)  # K=128 warmup operand + bcast ones
    nc.vector.memset(wu_sb, 1.0)

    # ---- constant loads (hot-first emission order) -----------------------
    if QK_FP8:
        wq_sb = consts.tile([128, KT // 2, 2, 2, 128], FP8)
        nc.sync.dma_start(out=wq_sb, in_=io["wq"])
        xt8_sb = consts.tile([128, KT // 2, 2, S], FP8)
        # Activation-engine DMA queue: runs parallel to the sync-queue
        # weight loads, halving the head-of-kernel input latency
        nc.scalar.dma_start(out=xt8_sb[:, :, :, 0:CH], in_=io["xt8c0"])
        wk_sb = consts.tile([128, KT // 2, 2, 2, 128], FP8)
        nc.sync.dma_start(out=wk_sb, in_=io["wk"])
    else:
        wq_sb = consts.tile([128, KT, DG], MM_DT)
        nc.sync.dma_start(out=wq_sb, in_=io["wq"])
        wk_sb = consts.tile([128, KT, DG], MM_DT)
        nc.sync.dma_start(out=wk_sb, in_=io["wk"])
        xt8_sb = None
    xt_sb = consts.tile([128, KT, S], XT_DT)
    nc.scalar.dma_start(out=xt_sb[:, :, 0:CH], in_=io["xtc0"])
    bq_sb = consts.tile([128, 2], F32)
    nc.sync.dma_start(out=bq_sb, in_=io["bq"])
    bk_sb = consts.tile([128, 2], F32)
    nc.sync.dma_start(out=bk_sb, in_=io["bk"])
    wv_sb = consts.tile([128, KT, DG], MM_DT)
    nc.sync.dma_start(out=wv_sb, in_=io["wv"])
    vb_sb = consts.tile([128, HPG, HD], F32)
    nc.sync.dma_start(out=vb_sb, in_=io["vb"])
    if QK_FP8:
        nc.sync.dma_start(out=xt8_sb[:, :, :, CH:S], in_=io["xt8r"])
    nc.sync.dma_start(out=xt_sb[:, :, CH:S], in_=io["xtr"])
    wo_sb = consts.tile([128, 2, 1024], MM_DT)
    nc.sync.dma_start(out=wo_sb, in_=io["wo"])

    # ---- persistent activations ----------------------------------------
    qt_sb = acts.tile([128, 2, S], QK_SB_DT)  # Q^T (pre-scaled by 1/8 via host W/b)
    kt_sb = acts.tile([128, 2, S], QK_SB_DT)  # K^T
    v_sb = acts.tile([128, ST, HPG, HD], MM_DT)  # V blocks
    ctxT_sb = acts.tile([128, 2, S], MM_DT)  # normalized ctx^T

    # ctx accumulator: pair pr in bank pr, head 2pr+sub at partitions 64*sub
    # (matches ctxT layout exactly); denominators col-tiled at partition 32h
    ctx2_ps = psum.tile([128, 2, CH], F32, tag="ctx", bufs=1, name="ctx2_ps")
    den_ps = psum.tile([128, CH], F32, tag="denp", bufs=1, name="den_ps")

    # ---- PE warmup: full-K matmuls flip the HAM clock gate to 8/8 -------
    wu_ps = psum.tile([128, WU_N], F32, tag="sT", bufs=2, name="wu_ps")
    for r in range(WARMUP):
        nc.tensor.matmul(
            wu_ps, lhsT=wu_sb, rhs=wu_sb[:, 0:WU_N], start=True, stop=True
        )

    def proj_chains(c):
        """Q^T/K^T chunk c + V s-tiles of chunk c as a list of emit-thunks."""
        chains = []
        for t in range(2):
            for which, w_sb, b_sb, dst in (
                ("q", wq_sb, bq_sb, qt_sb),
                ("k", wk_sb, bk_sb, kt_sb),
            ):

                def chain(t=t, w_sb=w_sb, b_sb=b_sb, dst=dst, which=which):
                    ps = psum.tile([128, CH], F32, tag="sT", bufs=2, name=f"{which}_ps{c}{t}")
                    if QK_FP8:
                        for j in range(KT // 2):
                            nc.tensor.matmul(
                                ps[:, 0:CH],
                                lhsT=w_sb[:, j, t],
                                rhs=xt8_sb[:, j, :, ts(c, CH)],
                                start=(j == 0),
                                stop=(j == KT // 2 - 1),
                                perf_mode=mybir.MatmulPerfMode.DoubleRow,
                            )
                    else:
                        for kt in range(KT):
                            nc.tensor.matmul(
                                ps[:, 0:CH],
                                lhsT=w_sb[:, kt, ts(t, 128)],
                                rhs=xt_sb[:, kt, ts(c, CH)],
                                start=(kt == 0),
                                stop=(kt == KT - 1),
                            )
                    nc.scalar.activation(
                        out=dst[:, t, ts(c, CH)], in_=ps[:, 0:CH],
                        func=Identity, bias=b_sb[:, t : t + 1],
                        scale=(((SQ8 if which == "q" else SK8) if SC_FP8 else 1.0)
                               / (SCQ if which == "q" else SCK))
                        if QK_FP8 else 1.0,
                    )

                chains.append(chain)
        for st in range(4 * c, 4 * c + 4):

            def chain(st=st):
                ps = psum.tile([128, DG], F32, tag="sT", bufs=2, name=f"v_ps{st}")
                for kt in range(KT):
                    nc.tensor.matmul(
                        ps,
                        lhsT=xt_sb[:, kt, ts(st, 128)],
                        rhs=wv_sb[:, kt, :],
                        start=(kt == 0),
                        stop=(kt == KT - 1),
                    )
                nc.vector.tensor_add(
                    out=v_sb[:, st, :, :],
                    in0=ps.rearrange("p (h j) -> p h j", h=HPG),
                    in1=vb_sb,
                )

            chains.append(chain)
        return chains

    def proj_pieces(c):
        """proj chains for chunk c as ~1us emit-thunks on the aux psum bank
        (never touches the sT tag, so score double-buffering is untouched)."""
        pieces = []
        for t in range(2):
            for which, w_sb, b_sb, dst in (
                ("q", wq_sb, bq_sb, qt_sb),
                ("k", wk_sb, bk_sb, kt_sb),
            ):
                shared = {}

                def p1(t=t, w_sb=w_sb, shared=shared, which=which):
                    ps = psum.tile(
                        [128, CH], F32, tag="aux", bufs=1, name=f"{which}_ps"
                    )
                    shared["ps"] = ps
                    if QK_FP8:
                        for j in range(2):
                            nc.tensor.matmul(
                                ps,
                                lhsT=w_sb[:, j, t],
                                rhs=xt8_sb[:, j, :, ts(c, CH)],
                                start=(j == 0),
                                stop=False,
                                perf_mode=mybir.MatmulPerfMode.DoubleRow,
                            )
                    else:
                        for kt in range(4):
                            nc.tensor.matmul(
                                ps,
                                lhsT=w_sb[:, kt, ts(t, 128)],
                                rhs=xt_sb[:, kt, ts(c, CH)],
                                start=(kt == 0),
                                stop=False,
                            )

                def p2(t=t, w_sb=w_sb, b_sb=b_sb, dst=dst, shared=shared, which=which):
                    ps = shared["ps"]
                    if QK_FP8:
                        for j in range(2, KT // 2):
                            nc.tensor.matmul(
                                ps,
                                lhsT=w_sb[:, j, t],
                                rhs=xt8_sb[:, j, :, ts(c, CH)],
                                start=False,
                                stop=(j == KT // 2 - 1),
                                perf_mode=mybir.MatmulPerfMode.DoubleRow,
                            )
                    else:
                        for kt in range(4, KT):
                            nc.tensor.matmul(
                                ps,
                                lhsT=w_sb[:, kt, ts(t, 128)],
                                rhs=xt_sb[:, kt, ts(c, CH)],
                                start=False,
                                stop=(kt == KT - 1),
                            )
                    nc.scalar.activation(
                        out=dst[:, t, ts(c, CH)], in_=ps,
                        func=Identity, bias=b_sb[:, t : t + 1],
                        scale=(((SQ8 if which == "q" else SK8) if SC_FP8 else 1.0)
                               / (SCQ if which == "q" else SCK))
                        if QK_FP8 else 1.0,
                    )

                pieces += [p1, p2]
        for st in range(4 * c, 4 * c + 4):

            def pv(st=st):
                ps = psum.tile([128, DG], F32, tag="aux", bufs=1, name=f"v_ps{st}")
                for kt in range(KT):
                    nc.tensor.matmul(
                        ps,
                        lhsT=xt_sb[:, kt, ts(st, 128)],
                        rhs=wv_sb[:, kt, :],
                        start=(kt == 0),
                        stop=(kt == KT - 1),
                    )
                nc.vector.tensor_add(
                    out=v_sb[:, st, :, :],
                    in0=ps.rearrange("p (h j) -> p h j", h=HPG),
                    in1=vb_sb,
                )

            pieces.append(pv)
        return pieces

    def oproj_pieces(c):
        """out-projection for chunk c as 2 thunks per s-tile (aux psum)."""
        pieces = []
        for st in range(4 * c, 4 * c + 4):
            shared = {}
            for nch in range(2):

                def p(st=st, shared=shared, c=c, nch=nch):
                    if nch == 0:
                        shared["o_sb"] = work.tile(
                            [128, 1024], MM_DT, tag="osb", bufs=3, name="o_sb"
                        )
                    o_sb = shared["o_sb"]
                    if c == QCH - 1:
                        # tail: the sT banks are free after the last exp
                        ps = psum.tile([128, CH], F32, tag="sT", bufs=2, name="o_ps")
                    else:
                        ps = psum.tile([128, CH], F32, tag="aux", bufs=1, name="o_ps")
                    for t in range(2):
                        nc.tensor.matmul(
                            ps,
                            lhsT=ctxT_sb[:, t, ts(st, 128)],
                            rhs=wo_sb[:, t, ts(nch, CH)],
                            start=(t == 0),
                            stop=(t == 1),
                        )
                    # out evictions split across both psum-capable engines
                    if nch == 0:
                        nc.scalar.copy(out=o_sb[:, ts(nch, CH)], in_=ps)
                    else:
                        nc.vector.tensor_copy(out=o_sb[:, ts(nch, CH)], in_=ps)
                    if nch == 1:
                        nc.sync.dma_start(out=io["out"][ts(st, 128), :], in_=o_sb)

                pieces.append(p)
        return pieces

    def emit_attn(c, fillers=(), prs=(0, 1)):
        """S^T/exp per head-pair, ctx matmuls trailing by PIPE k-steps;
        filler thunks are drip-fed between tiles to soak up the PE slack
        under the exp-paced ScalarE stream.  prs restricts to a subset of
        head pairs (chunk 0 runs pair 0 before the t=1 projections land)."""
        from collections import deque

        fl = deque(fillers)
        nkt = (c + 1) * (CH // 128)
        exps = [[None] * nkt for _ in range(2)]  # per pair

        def scores(i):
            off = max(0, 128 * i - CH * c)  # first unmasked column of this k-tile
            for pr in prs:  # head pair (2*pr, 2*pr+1) -> tile t=pr
                sT_ps = psum.tile([128, 2, CH], F32, tag="sT", bufs=2, name="sT_ps")
                for sub in range(2):
                    if SC_FP8:
                        # DoubleRow: head h's 64 dims live as [32p, 2slot];
                        # strips 32h give 4 distinct row-tile positions
                        h = 2 * pr + sub
                        nc.tensor.matmul(
                            sT_ps[:, sub, off:CH],
                            lhsT=kt_sb[32 * h : 32 * h + 32, :, ts(i, 128)],
                            rhs=qt_sb[32 * h : 32 * h + 32, :, c * CH + off : (c + 1) * CH],
                            start=True,
                            stop=True,
                            perf_mode=mybir.MatmulPerfMode.DoubleRow,
                            tile_position=(32 * h, 0),
                        )
                    else:
                        pb = sub * 64
                        nc.tensor.matmul(
                            sT_ps[:, sub, off:CH],
                            lhsT=kt_sb[pb : pb + HD, pr, ts(i, 128)],
                            rhs=qt_sb[pb : pb + HD, pr, c * CH + off : (c + 1) * CH],
                            start=True,
                            stop=True,
                        )
                ei = work.tile([128, 2, CH], I16, tag="exp", bufs=10, name="e")
                e = ei.bitcast(BF16)
                # the two head-pairs' exps run CONCURRENTLY on different
                # engines: pr0 = ScalarE table exp, pr1 = DVE Schraudolph
                # (bf16 exp bits via fma + f32->i16 round); both ~1.1us, so
                # the k-tile exp pace halves vs one engine doing both.
                dsc = 1.0 / (SQ8 * SK8) if SC_FP8 else 1.0
                if pr == 0:
                    nc.scalar.activation(
                        out=e[:, :, off:CH], in_=sT_ps[:, :, off:CH], func=Exp,
                        scale=dsc,
                    )
                else:
                    nc.vector.tensor_scalar(
                        out=ei[:, :, off:CH],
                        in0=sT_ps[:, :, off:CH],
                        scalar1=A16 * dsc,
                        scalar2=B16,
                        op0=mybir.AluOpType.mult,
                        op1=mybir.AluOpType.add,
                    )
                if 128 * i + 128 > CH * c + off:  # crosses the diagonal: mask
                    # only the 128-wide crossing block needs masking;
                    # columns beyond it are fully unmasked
                    mw = min(128, CH - off)
                    nc.gpsimd.affine_select(
                        out=e[:, :, off : off + mw],
                        in_=e[:, :, off : off + mw],
                        pattern=[[0, 2], [1, mw]],
                        base=0,
                        channel_multiplier=-1,
                        compare_op=mybir.AluOpType.is_ge,
                        fill=0.0,
                    )
                exps[pr][i] = (e, off)

        def ctx(i):
            # 2 heads col-tiled per bank (concurrent on array col halves);
            # start=True clears has_written per written region, so each
            # head's first matmul of the chunk needs it
            for pr in prs:
                e, off = exps[pr][i]
                for sub in range(2):
                    nc.tensor.matmul(
                        ctx2_ps[64 * sub : 64 * sub + 64, pr, off:CH],
                        lhsT=v_sb[:, i, 2 * pr + sub, :],
                        rhs=e[:, sub, off:CH],
                        start=(i == 0),
                        stop=(i == nkt - 1),
                        tile_position=(0, 64 * sub),
                    )
            # denominators: 4-way col-tiled M=1 matmuls, head h at partition 32h.
            # For paired k-tiles (both off=0), DVE pre-sums e(i-1)+e(i) and one
            # den wave covers both; otherwise one wave per k-tile.
            paired = DEN_PAIR and i < 4 * c and i % 2 == 1
            skip = DEN_PAIR and i < 4 * c and i % 2 == 0
            if skip:
                return
            first_den = (i == 0) or (DEN_PAIR and c > 0 and i == 1)
            if paired:
                rhs_by_pr = {}
                for pr in prs:
                    e_a, _ = exps[pr][i - 1]
                    e_b, _ = exps[pr][i]
                    es = work.tile([128, 2, CH], BF16, tag="esum", bufs=2, name="esum")
                    nc.vector.tensor_add(out=es, in0=e_a, in1=e_b)
                    rhs_by_pr[pr] = es
            else:
                rhs_by_pr = {pr: exps[pr][i][0] for pr in prs}
            for pr in prs:
                e, off = exps[pr][i]
                if paired:
                    e, off = rhs_by_pr[pr], 0
                for sub in range(2):
                    h = 2 * pr + sub
                    nc.tensor.matmul(
                        den_ps[32 * h : 32 * h + 1, off:CH],
                        lhsT=wu_sb[:, 0:1],
                        rhs=e[:, sub, off:CH],
                        start=first_den,
                        stop=(i == nkt - 1),
                        tile_position=(0, 32 * h),
                    )

        steps = nkt + PIPE
        for i in range(steps):
            if i < nkt:
                scores(i)
            if i >= PIPE:
                ctx(i - PIPE)
            if fl:
                k = min(2, max(1, -(-len(fl) // (steps - i))))
                for _ in range(k):
                    if fl:
                        fl.popleft()()
        while fl:
            fl.popleft()()

    def tail_norm_parts(c):
        """Denominator eviction + normalize thunks for chunk c."""
        parts = []
        den_sb = small.tile([97, CH], MM_DT, tag="den", name="den_sb")

        def den_evict():
            # denominators live at partitions 0/32/64/96 of den_ps; evict the
            # whole 97-row band in one op (ScalarE: DVE is exp-loaded)
            nc.scalar.copy(out=den_sb, in_=den_ps[0:97, :])

        parts.append(den_evict)
        for pr in range(2):

            def norm(pr=pr, c=c):
                bc_ps = psum.tile([128, CH], F32, tag="sT", bufs=2, name="bc_ps")
                for sub in range(2):
                    p = 32 * (2 * pr + sub)  # weight and fmap must share start partition
                    nc.tensor.matmul(
                        bc_ps[64 * sub : 64 * sub + 64, :],
                        lhsT=wu_sb[p : p + 1, 0:HD],
                        rhs=den_sb[p : p + 1, :],
                        start=True,
                        stop=True,
                        tile_position=(p, 64 * sub),
                    )
                rcp_sb = small.tile([128, CH], F32, tag="rcp", name="rcp_sb")
                nc.vector.reciprocal_approx_fast(out=rcp_sb, in_=bc_ps)
                nc.vector.tensor_mul(
                    out=ctxT_sb[:, pr, ts(c, CH)],
                    in0=ctx2_ps[:, pr, :],
                    in1=rcp_sb,
                )

            parts.append(norm)
        return parts

    def run_all(thunks):
        for th in thunks:
            th()

    # ---- pipeline: projections + out-projections drip into the exp-paced
    # attention stream; only the norm chain sits between phases -----------
    # piece layout: [0:8] = Q/K chain halves, [8:12] = V chains.  A chunk's
    # V tiles are first consumed by its OWN attention (ctx(c, 4c..) at step
    # 4c+PIPE), so V(c) rides at the front of attn(c)'s fillers — this
    # drains the overloaded early phases and feeds the starved attn(3).
    pp1, pp2, pp3 = proj_pieces(1), proj_pieces(2), proj_pieces(3)

    def wu_fill(n):
        def f():
            ps = psum.tile([128, WU_N], F32, tag="aux", bufs=1, name="wf_ps")
            for _ in range(n):
                nc.tensor.matmul(
                    ps, lhsT=wu_sb, rhs=wu_sb[:, 0:WU_N], start=True, stop=True
                )

        return f

    run_all(proj_chains(0))
    emit_attn(0, pp1[0:8])
    # each chunk's norm chain rides as the FIRST fillers of the next chunk's
    # attention: den-evict/bc/recip/mul land in steps 0-2, before ctx(c+1,0)
    # at step PIPE needs the ctx2/den psum banks back -- no boundary stall
    emit_attn(1, tail_norm_parts(0) + pp1[8:12] + pp2[0:8])
    emit_attn(2, tail_norm_parts(1) + pp2[8:12] + oproj_pieces(0) + pp3[0:8])
    # chunk 3 has no next-chunk projections to drip, so it takes both
    # trailing out-projection chunks instead of junk filler matmuls
    emit_attn(3, tail_norm_parts(2) + pp3[8:12] + oproj_pieces(1) + oproj_pieces(2))
    # final norm: den-evict first, filler burst covers the bc/recip/mul
    # latency so the PE stays warm until the tail out-projections' inputs
    parts = tail_norm_parts(3)
    parts[0]()
    tf_ps = psum.tile([128, WU_N], F32, tag="sT", bufs=2, name="tf_ps")
    for r in range(TAILFILL):
        nc.tensor.matmul(
            tf_ps, lhsT=wu_sb, rhs=wu_sb[:, 0:WU_N], start=True, stop=True
        )
    run_all(parts[1:])
    run_all(oproj_pieces(3))

    psum.release()
    small.release()
    work.release()
    acts.release()
    consts.release()


_LDW_PATCHED = False


def _maybe_enable_ldw_opt():
    """Optionally flip walrus's --enable-ldw-opt (fast weight load)."""
    global _LDW_PATCHED
    if _LDW_PATCHED or os.environ.get("BASS_LDW_OPT", "0") != "1":
        return
    import concourse.bass_utils as bu

    orig = bu.run_command

    def run_command(cmd, *a, **kw):
        cmd = [
            "--enable-ldw-opt=true" if c == "--enable-ldw-opt=false" else c
            for c in cmd
        ]
        return orig(cmd, *a, **kw)

    bu.run_command = run_command
    _LDW_PATCHED = True


def build_nc():
    _maybe_enable_ldw_opt()
    nc = bacc.Bacc("TRN2", target_bir_lowering=False, debug=False)
    io = {
        "xtc0": nc.dram_tensor("xtc0", [128, KT, CH], XT_DT, kind="ExternalInput").ap(),
        "xtr": nc.dram_tensor("xtr", [128, KT, S - CH], XT_DT, kind="ExternalInput").ap(),
        "wv": nc.dram_tensor("wv", [128, KT, DG], MM_DT, kind="ExternalInput").ap(),
    }
    if QK_FP8:
        io["wq"] = nc.dram_tensor("wq", [128, KT // 2, 2, 2, 128], FP8, kind="ExternalInput").ap()
        io["wk"] = nc.dram_tensor("wk", [128, KT // 2, 2, 2, 128], FP8, kind="ExternalInput").ap()
        io["xt8c0"] = nc.dram_tensor("xt8c0", [128, KT // 2, 2, CH], FP8, kind="ExternalInput").ap()
        io["xt8r"] = nc.dram_tensor("xt8r", [128, KT // 2, 2, S - CH], FP8, kind="ExternalInput").ap()
    else:
        io["wq"] = nc.dram_tensor("wq", [128, KT, DG], MM_DT, kind="ExternalInput").ap()
        io["wk"] = nc.dram_tensor("wk", [128, KT, DG], MM_DT, kind="ExternalInput").ap()
    io.update({
        "wo": nc.dram_tensor("wo", [128, 2, 1024], MM_DT, kind="ExternalInput").ap(),
        "bq": nc.dram_tensor("bq", [128, 2], F32, kind="ExternalInput").ap(),
        "bk": nc.dram_tensor("bk", [128, 2], F32, kind="ExternalInput").ap(),
        "vb": nc.dram_tensor("vb", [128, HPG, HD], F32, kind="ExternalInput").ap(),
        "out": nc.dram_tensor("out", [S, D], MM_DT, kind="ExternalOutput").ap(),
    })
    with tile.TileContext(nc) as tc, nc.allow_low_precision(
        reason="reduced-precision matmul operand pipeline; accumulation stays fp32"
    ):
        build_kernel_body(nc, tc, io)
    nc.compile()
    return nc


_NC = None


def get_nc():
    global _NC
    if _NC is None:
        _NC = build_nc()
    return _NC


def _tile_rows(a, p=128, dt=None):
    """[R, N] -> [128, R//128, N] with row r = kt*128 + p."""
    r, n = a.shape
    return np.ascontiguousarray(a.reshape(r // p, p, n).transpose(1, 0, 2)).astype(
        dt if dt is not None else _np_dt()
    )


def _xt_dt():
    import ml_dtypes

    return ml_dtypes.float8_e4m3fn if XT_FP8 else _np_dt()


def _pack_w8(W, sc):
    """[1024, 256] -> [128p, 4j, 2t, 2s, 128m] fp8 (DoubleRow pairs of k-tiles)."""
    import ml_dtypes

    return np.ascontiguousarray(
        (W * sc).reshape(4, 2, 128, 2, 128).transpose(2, 0, 3, 1, 4)
    ).astype(ml_dtypes.float8_e4m3fn)


def _pack_x8(xT):
    """[1024, 2048] -> [128p, 4j, 2s, 2048] fp8."""
    import ml_dtypes

    return np.ascontiguousarray(
        xT.reshape(4, 2, 128, S).transpose(2, 0, 1, 3)
    ).astype(ml_dtypes.float8_e4m3fn)


def shard_inputs(x, Wq, bq, Wk, bk, Wv, bv, Wo, bo):
    scale = 1.0 / np.sqrt(np.float32(HD))
    in_maps = []
    for core in range(8):
        b, g = divmod(core, GROUPS)
        sl = slice(g * DG, (g + 1) * DG)
        vb = np.ascontiguousarray(
            np.broadcast_to(bv[sl].reshape(HPG, HD)[None], (128, HPG, HD))
        ).astype(np.float32)
        xT = np.ascontiguousarray(x[b].T)
        if QK_FP8:
            x8 = _pack_x8(xT)
            Wq_s, Wk_s = Wq[:, sl] * scale, np.asarray(Wk[:, sl])
            bq_s, bk_s = bq[sl] * scale, np.asarray(bk[sl])
            if SC_FP8:
                # head-strip column order: slot s holds dims [32s,32s+32)
                # of each head; biases pre-scaled by the fp8 range factor
                perm = np.array(
                    [h * 64 + s_ * 32 + p for s_ in range(2) for h in range(4) for p in range(32)]
                )
                Wq_s, Wk_s = Wq_s[:, perm], Wk_s[:, perm]
                bq_s, bk_s = bq_s[perm] * SQ8, bk_s[perm] * SK8
            qk = {
                "wq": _pack_w8(np.ascontiguousarray(Wq_s), SCQ),
                "wk": _pack_w8(np.ascontiguousarray(Wk_s), SCK),
                "xt8c0": np.ascontiguousarray(x8[:, :, :, 0:CH]),
                "xt8r": np.ascontiguousarray(x8[:, :, :, CH:S]),
            }
        else:
            qk = {
                "wq": _tile_rows(np.ascontiguousarray(Wq[:, sl]) * scale),
                "wk": _tile_rows(np.ascontiguousarray(Wk[:, sl])),
            }
        xt = _tile_rows(xT, dt=_xt_dt())
        in_maps.append(
            {
                **qk,
                "xtc0": np.ascontiguousarray(xt[:, :, 0:CH]),
                "xtr": np.ascontiguousarray(xt[:, :, CH:S]),
                "wv": _tile_rows(np.ascontiguousarray(Wv[:, sl])),
                "wo": _tile_rows(np.ascontiguousarray(Wo[sl, :])),
                "bq": np.ascontiguousarray(
                    (bq_s if QK_FP8 else bq[sl] * scale).reshape(2, 128).T
                ).astype(np.float32),
                "bk": np.ascontiguousarray(
                    (bk_s if QK_FP8 else bk[sl]).reshape(2, 128).T
                ).astype(np.float32),
                "vb": vb,
            }
        )
    return in_maps


LAST_RESULT = None


def kernel(**inputs):
    global LAST_RESULT
    inputs = {k: np.asarray(v) for k, v in inputs.items()}
    nc = get_nc()
    in_maps = shard_inputs(**inputs)
    trace = bool(int(os.environ.get("BASS_KERNEL_TRACE", "0")))
    res = run_bass_kernel_spmd(nc, in_maps, core_ids=list(range(8)), trace=trace)
    LAST_RESULT = res
    parts = [res.results[c]["out"].astype(np.float32) for c in range(8)]
    out = np.stack(
        [
            parts[0] + parts[1] + parts[2] + parts[3],
            parts[4] + parts[5] + parts[6] + parts[7],
        ]
    )
    return (out + inputs["bo"]).astype(np.float32)



# revision 55
# speedup vs baseline: 1.0314x; 1.0314x over previous
"""TRN2 Bass/Tile kernel: 16-head causal multi-head attention.

Problem: x[2,2048,1024], 16 heads x 64, causal softmax attention + out-proj.

Sharding (8 cores): core = b*4 + g  (b = batch 0..1, g = head-group 0..3).
Each core computes heads [4g, 4g+4) for batch b and the partial
out-projection  ctx_g @ Wo[g*256:(g+1)*256, :]  -> [2048, 1024] (bf16).
Host upcasts, sums the 4 partials per batch and adds bo.

On-device layout is fully "transposed" (feature-major):
  xT   [128, 8, 2048]  : xT[p, kt, s]  = x[b, s, kt*128+p]
  QT/KT[128, 2, 2048]  : QT[p, t, s]   = Q^T[t*128+p, s]   (d' = h*64+j on partitions)
  S^T  [128k, 512q]    : per (head, q-chunk, k-tile) block = K @ Q^T
  softmax: no max-subtraction (scores are O(1) by construction: exp is safe);
  denominators via a ones-column appended to V (row 64 of the ctx psum);
  ctx^T [128, 2, 2048] feeds the out-projection directly as lhsT.

Schedule (v16, from v9):
  - Dual-engine exp: each k-tile's two head-pair exps run CONCURRENTLY --
    pr0 as exact table exp on ScalarE, pr1 as a Schraudolph bit-trick exp
    on DVE (one tensor_scalar: bf16 exp bits = round(x*128/ln2 + 16250),
    f32->int16 convert-on-write, tile bitcast int16->bf16; ~3.4% max rel
    err, ~0.014 end-to-end).  This halves the old ScalarE exp pacing.
  - fp8e4m3 DoubleRow Q/K projections: host packs W/x into [p, j, 2, .]
    k-tile pairs; 4 DR matmuls (K=256 each) replace 8 bf16 ones at the
    same per-instruction cadence (2x PE throughput).  Host scales W into
    fp8 range (SCQ/SCK); the eviction's activation descales.  V/Wo stay
    bf16 (fp8 there fails the 2e-2 gate: early causal rows pass V errors
    straight through).
  - PSUM evictions split across both psum-capable engines: o_sb nch0 +
    Q/K bias (Identity+bias AP) + den band on ScalarE; o_sb nch1 + V bias
    add + norm on DVE.
  - Each chunk's norm chain (den evict/bcast/recip/mul) rides as the
    first fillers of the NEXT chunk's attention; ctx(c+1) only needs the
    banks back at step PIPE=3, so chunk boundaries don't stall the PE.
  - Contiguous chunk-0/rest DMA splits for xt/xt8 (strided per-k-tile
    loads ran ~2x slower and stalled the first projection chain); the
    chunk-0 x loads issue from the Activation engine's DMA queue, running
    parallel to the sync-queue weight loads (only safe while ScalarE is
    idle -- mid-kernel issue ops would block its eviction stream).
  - Drip-load balance: chunk 3 (no next-chunk projections) carries both
    trailing out-projection chunks; warmup sized (56) to the parallel
    head-DMA latency.

Schedule (v9):
  - Head pairs (h0,h1)/(h2,h3) write one [128, 2, 512] two-bank S^T psum
    tile; their matmuls row-tile the PE array concurrently (K=64 halves)
    and ONE exp per pair halves ScalarE's per-op fixed cost.
  - ctx: 2 heads col-tiled per psum bank (concurrent on array column
    halves, M=64 each, layout matches ctxT exactly -> no partition-shift
    bounce); denominators as 4-way col-tiled M=1 matmuls into one bank.
  - Normalization: den band -> SBUF (1 op, bf16), bf16 ones-matmul
    broadcasts per pair (col-tiled), reciprocal_approx_fast on 128
    lanes, one DVE mul per pair.  No Log/Exp table switches.
  - The attention phase is exp-paced (ScalarE ~2.2us/k-tile vs ~1.2us of
    PE work); the next chunk's projections and the previous chunk's
    out-projections are split into ~1us pieces on a dedicated aux psum
    bank and drip-fed between k-tiles so the in-order PE queue can chew
    them under the exp stream.
  - Causal mask (affine_select) trimmed to the 128-wide crossing block.
  - Output stored bf16 (halves the 8MB out DMA); host sums in f32.
  - Warmup: full-K N=128 matmuls from memset tiles (no DMA dep) flip the
    PE HAM clock gate to 8/8 through the initial DMA; filler matmuls
    bridge the final norm latency before the tail out-projections.
"""

import os
import sys

for _p in ("/opt/trn_rl_repo",):
    if _p not in sys.path:
        sys.path.insert(0, _p)

import numpy as np

import concourse.bass as bass
import concourse.mybir as mybir
import concourse.tile as tile
from concourse import bacc
from concourse.bass import ts
from concourse.bass_utils import run_bass_kernel_spmd

B, S, D, H, HD = 2, 2048, 1024, 16, 64
GROUPS, HPG, DG = 4, 4, 256  # head groups, heads/group, group width
KT = D // 128  # 8 k-tiles over D
ST = S // 128  # 16 s-tiles
CH = 512  # q-chunk width
QCH = S // CH  # 4 q-chunks
PIPE = int(os.environ.get("BASS_PIPE", "3"))  # ctx trails S^T/exp by this many k-steps
F32 = mybir.dt.float32
I16 = mybir.dt.int16
BF16 = mybir.dt.bfloat16

# Schraudolph exp on DVE: bf16 bits = round(A16*x + (16256 - C16)); bitcast
# int16 -> bf16 gives exp(x) with ~3.4% max rel err (0.0097 end-to-end).
A16 = 128.0 / float(np.log(2.0))
C16 = float(os.environ.get("BASS_C16", "6"))
B16 = 16256.0 - C16
# every SEXP_MOD-th exp op stays on ScalarE (table exp); 0 = all on DVE
SEXP_MOD = int(os.environ.get("BASS_SEXP_MOD", "5"))

_MM_DT_NAME = os.environ.get("BASS_MM_DT", "bf16")
MM_DT = {
    "f32r": mybir.dt.float32r,
    "f32": mybir.dt.float32,
    "bf16": mybir.dt.bfloat16,
}[_MM_DT_NAME]
WARMUP = int(os.environ.get("BASS_WARMUP", "56"))
# fp8e4m3 DoubleRow for the Q/K projections: 2 k-tiles per pass, 2x PE rate.
QK_FP8 = os.environ.get("BASS_QK_FP8", "1") == "1"
FP8 = mybir.dt.float8e4
SCQ = 512.0  # host: Wq*(1/8)*SCQ into fp8; descaled at eviction
SCK = 64.0
# fp8 Q/K SBUF tiles + DoubleRow scores (2 heads per pair at 32-row strips):
# halves the scores stream. Host permutes W columns into head-strip order
# (slot s = dims [32s,32s+32) of each head); the exp descales by 1/(SQ8*SK8).
SC_FP8 = QK_FP8 and os.environ.get("BASS_SC_FP8", "0") == "1"
SQ8 = 8.0  # Q scaled into fp8 range at eviction; exp descales
SK8 = 1.0
QK_SB_DT = FP8 if SC_FP8 else MM_DT
XT_FP8 = os.environ.get("BASS_XT_FP8", "0") == "1"
XT_DT = mybir.dt.float8e4 if XT_FP8 else MM_DT
TAILFILL = int(os.environ.get("BASS_TAILFILL", "12"))
# pair non-crossing k-tiles' denominators: DVE pre-sums e(2m)+e(2m+1),
# one PE den wave per pair instead of two (PE is the wall, DVE has slack)
DEN_PAIR = os.environ.get("BASS_DEN_PAIR", "0") == "1"
WU_N = int(os.environ.get("BASS_WU_N", "128"))


def _np_dt():
    import ml_dtypes

    return ml_dtypes.bfloat16 if _MM_DT_NAME == "bf16" else np.float32


def build_kernel_body(nc, tc, io):
    Exp = mybir.ActivationFunctionType.Exp
    Identity = mybir.ActivationFunctionType.Identity
    exp_ctr = [0]

    consts = tc.alloc_tile_pool(name="consts", bufs=1)
    acts = tc.alloc_tile_pool(name="acts", bufs=1)
    work = tc.alloc_tile_pool(name="work", bufs=2)
    small = tc.alloc_tile_pool(name="small", bufs=2)
    psum = tc.alloc_tile_pool(name="psum", bufs=1, space="PSUM")

    # ---- on-chip constants (no DMA dependency: warmup starts at t=0) ----
    wu_sb = consts.tile([128, 128], MM_DT)  # K=128 warmup operand + bcast ones
    nc.vector.memset(wu_sb, 1.0)

    # ---- constant loads (hot-first emission order) -----------------------
    if QK_FP8:
        wq_sb = consts.tile([128, KT // 2, 2, 2, 128], FP8)
        nc.sync.dma_start(out=wq_sb, in_=io["wq"])
        xt8_sb = consts.tile([128, KT // 2, 2, S], FP8)
        # Activation-engine DMA queue: runs parallel to the sync-queue
        # weight loads, halving the head-of-kernel input latency
        nc.scalar.dma_start(out=xt8_sb[:, :, :, 0:CH], in_=io["xt8c0"])
        wk_sb = consts.tile([128, KT // 2, 2, 2, 128], FP8)
        nc.sync.dma_start(out=wk_sb, in_=io["wk"])
    else:
        wq_sb = consts.tile([128, KT, DG], MM_DT)
        nc.sync.dma_start(out=wq_sb, in_=io["wq"])
        wk_sb = consts.tile([128, KT, DG], MM_DT)
        nc.sync.dma_start(out=wk_sb, in_=io["wk"])
        xt8_sb = None
    xt_sb = consts.tile([128, KT, S], XT_DT)
    nc.scalar.dma_start(out=xt_sb[:, :, 0:CH], in_=io["xtc0"])
    bq_sb = consts.tile([128, 2], F32)
    nc.sync.dma_start(out=bq_sb, in_=io["bq"])
    bk_sb = consts.tile([128, 2], F32)
    nc.sync.dma_start(out=bk_sb, in_=io["bk"])
    wv_sb = consts.tile([128, KT, DG], MM_DT)
    nc.sync.dma_start(out=wv_sb, in_=io["wv"])
    vb_sb = consts.tile([128, HPG, HD], F32)
    nc.sync.dma_start(out=vb_sb, in_=io["vb"])
    if QK_FP8:
        nc.sync.dma_start(out=xt8_sb[:, :, :, CH:S], in_=io["xt8r"])
    nc.sync.dma_start(out=xt_sb[:, :, CH:S], in_=io["xtr"])
    wo_sb = consts.tile([128, 2, 1024], MM_DT)
    nc.sync.dma_start(out=wo_sb, in_=io["wo"])

    # ---- persistent activations ----------------------------------------
    qt_sb = acts.tile([128, 2, S], QK_SB_DT)  # Q^T (pre-scaled by 1/8 via host W/b)
    kt_sb = acts.tile([128, 2, S], QK_SB_DT)  # K^T
    v_sb = acts.tile([128, ST, HPG, HD], MM_DT)  # V blocks
    ctxT_sb = acts.tile([128, 2, S], MM_DT)  # normalized ctx^T

    # ctx accumulator: pair pr in bank pr, head 2pr+sub at partitions 64*sub
    # (matches ctxT layout exactly); denominators col-tiled at partition 32h
    ctx2_ps = psum.tile([128, 2, CH], F32, tag="ctx", bufs=1, name="ctx2_ps")
    den_ps = psum.tile([128, CH], F32, tag="denp", bufs=1, name="den_ps")

    # ---- PE warmup: full-K matmuls flip the HAM clock gate to 8/8 -------
    wu_ps = psum.tile([128, WU_N], F32, tag="sT", bufs=2, name="wu_ps")
    for r in range(WARMUP):
        nc.tensor.matmul(
            wu_ps, lhsT=wu_sb, rhs=wu_sb[:, 0:WU_N], start=True, stop=True
        )

    def proj_chains(c):
        """Q^T/K^T chunk c + V s-tiles of chunk c as a list of emit-thunks."""
        chains = []
        for t in range(2):
            for which, w_sb, b_sb, dst in (
                ("q", wq_sb, bq_sb, qt_sb),
                ("k", wk_sb, bk_sb, kt_sb),
            ):

                def chain(t=t, w_sb=w_sb, b_sb=b_sb, dst=dst, which=which):
                    ps = psum.tile([128, CH], F32, tag="sT", bufs=2, name=f"{which}_ps{c}{t}")
                    if QK_FP8:
                        for j in range(KT // 2):
                            nc.tensor.matmul(
                                ps[:, 0:CH],
                                lhsT=w_sb[:, j, t],
                                rhs=xt8_sb[:, j, :, ts(c, CH)],
                                start=(j == 0),
                                stop=(j == KT // 2 - 1),
                                perf_mode=mybir.MatmulPerfMode.DoubleRow,
                            )
                    else:
                        for kt in range(KT):
                            nc.tensor.matmul(
                                ps[:, 0:CH],
                                lhsT=w_sb[:, kt, ts(t, 128)],
                                rhs=xt_sb[:, kt, ts(c, CH)],
                                start=(kt == 0),
                                stop=(kt == KT - 1),
                            )
                    nc.scalar.activation(
                        out=dst[:, t, ts(c, CH)], in_=ps[:, 0:CH],
                        func=Identity, bias=b_sb[:, t : t + 1],
                        scale=(((SQ8 if which == "q" else SK8) if SC_FP8 else 1.0)
                               / (SCQ if which == "q" else SCK))
                        if QK_FP8 else 1.0,
                    )

                chains.append(chain)
        for st in range(4 * c, 4 * c + 4):

            def chain(st=st):
                ps = psum.tile([128, DG], F32, tag="sT", bufs=2, name=f"v_ps{st}")
                for kt in range(KT):
                    nc.tensor.matmul(
                        ps,
                        lhsT=xt_sb[:, kt, ts(st, 128)],
                        rhs=wv_sb[:, kt, :],
                        start=(kt == 0),
                        stop=(kt == KT - 1),
                    )
                nc.vector.tensor_add(
                    out=v_sb[:, st, :, :],
                    in0=ps.rearrange("p (h j) -> p h j", h=HPG),
                    in1=vb_sb,
                )

            chains.append(chain)
        return chains

    def proj_pieces(c):
        """proj chains for chunk c as ~1us emit-thunks on the aux psum bank
        (never touches the sT tag, so score double-buffering is untouched)."""
        pieces = []
        for t in range(2):
            for which, w_sb, b_sb, dst in (
                ("q", wq_sb, bq_sb, qt_sb),
                ("k", wk_sb, bk_sb, kt_sb),
            ):
                shared = {}

                def p1(t=t, w_sb=w_sb, shared=shared, which=which):
                    ps = psum.tile(
                        [128, CH], F32, tag="aux", bufs=1, name=f"{which}_ps"
                    )
                    shared["ps"] = ps
                    if QK_FP8:
                        for j in range(2):
                            nc.tensor.matmul(
                                ps,
                                lhsT=w_sb[:, j, t],
                                rhs=xt8_sb[:, j, :, ts(c, CH)],
                                start=(j == 0),
                                stop=False,
                                perf_mode=mybir.MatmulPerfMode.DoubleRow,
                            )
                    else:
                        for kt in range(4):
                            nc.tensor.matmul(
                                ps,
                                lhsT=w_sb[:, kt, ts(t, 128)],
                                rhs=xt_sb[:, kt, ts(c, CH)],
                                start=(kt == 0),
                                stop=False,
                            )

                def p2(t=t, w_sb=w_sb, b_sb=b_sb, dst=dst, shared=shared, which=which):
                    ps = shared["ps"]
                    if QK_FP8:
                        for j in range(2, KT // 2):
                            nc.tensor.matmul(
                                ps,
                                lhsT=w_sb[:, j, t],
                                rhs=xt8_sb[:, j, :, ts(c, CH)],
                                start=False,
                                stop=(j == KT // 2 - 1),
                                perf_mode=mybir.MatmulPerfMode.DoubleRow,
                            )
                    else:
                        for kt in range(4, KT):
                            nc.tensor.matmul(
                                ps,
                                lhsT=w_sb[:, kt, ts(t, 128)],
                                rhs=xt_sb[:, kt, ts(c, CH)],
                                start=False,
                                stop=(kt == KT - 1),
                            )
                    nc.scalar.activation(
                        out=dst[:, t, ts(c, CH)], in_=ps,
                        func=Identity, bias=b_sb[:, t : t + 1],
                        scale=(((SQ8 if which == "q" else SK8) if SC_FP8 else 1.0)
                               / (SCQ if which == "q" else SCK))
                        if QK_FP8 else 1.0,
                    )

                pieces += [p1, p2]
        for st in range(4 * c, 4 * c + 4):

            def pv(st=st):
                ps = psum.tile([128, DG], F32, tag="aux", bufs=1, name=f"v_ps{st}")
                for kt in range(KT):
                    nc.tensor.matmul(
                        ps,
                        lhsT=xt_sb[:, kt, ts(st, 128)],
                        rhs=wv_sb[:, kt, :],
                        start=(kt == 0),
                        stop=(kt == KT - 1),
                    )
                nc.vector.tensor_add(
                    out=v_sb[:, st, :, :],
                    in0=ps.rearrange("p (h j) -> p h j", h=HPG),
                    in1=vb_sb,
                )

            pieces.append(pv)
        return pieces

    def oproj_pieces(c):
        """out-projection for chunk c as 2 thunks per s-tile (aux psum)."""
        pieces = []
        for st in range(4 * c, 4 * c + 4):
            shared = {}
            for nch in range(2):

                def p(st=st, shared=shared, c=c, nch=nch):
                    if nch == 0:
                        shared["o_sb"] = work.tile(
                            [128, 1024], MM_DT, tag="osb", bufs=3, name="o_sb"
                        )
                    o_sb = shared["o_sb"]
                    if c == QCH - 1:
                        # tail: the sT banks are free after the last exp
                        ps = psum.tile([128, CH], F32, tag="sT", bufs=2, name="o_ps")
                    else:
                        ps = psum.tile([128, CH], F32, tag="aux", bufs=1, name="o_ps")
                    for t in range(2):
                        nc.tensor.matmul(
                            ps,
                            lhsT=ctxT_sb[:, t, ts(st, 128)],
                            rhs=wo_sb[:, t, ts(nch, CH)],
                            start=(t == 0),
                            stop=(t == 1),
                        )
                    # out evictions split across both psum-capable engines
                    if nch == 0:
                        nc.scalar.copy(out=o_sb[:, ts(nch, CH)], in_=ps)
                    else:
                        nc.vector.tensor_copy(out=o_sb[:, ts(nch, CH)], in_=ps)
                    if nch == 1:
                        nc.sync.dma_start(out=io["out"][ts(st, 128), :], in_=o_sb)

                pieces.append(p)
        return pieces

    def emit_attn(c, fillers=(), prs=(0, 1)):
        """S^T/exp per head-pair, ctx matmuls trailing by PIPE k-steps;
        filler thunks are drip-fed between tiles to soak up the PE slack
        under the exp-paced ScalarE stream.  prs restricts to a subset of
        head pairs (chunk 0 runs pair 0 before the t=1 projections land)."""
        from collections import deque

        fl = deque(fillers)
        nkt = (c + 1) * (CH // 128)
        exps = [[None] * nkt for _ in range(2)]  # per pair

        def scores(i):
            off = max(0, 128 * i - CH * c)  # first unmasked column of this k-tile
            for pr in prs:  # head pair (2*pr, 2*pr+1) -> tile t=pr
                sT_ps = psum.tile([128, 2, CH], F32, tag="sT", bufs=2, name="sT_ps")
                for sub in range(2):
                    if SC_FP8:
                        # DoubleRow: head h's 64 dims live as [32p, 2slot];
                        # strips 32h give 4 distinct row-tile positions
                        h = 2 * pr + sub
                        nc.tensor.matmul(
                            sT_ps[:, sub, off:CH],
                            lhsT=kt_sb[32 * h : 32 * h + 32, :, ts(i, 128)],
                            rhs=qt_sb[32 * h : 32 * h + 32, :, c * CH + off : (c + 1) * CH],
                            start=True,
                            stop=True,
                            perf_mode=mybir.MatmulPerfMode.DoubleRow,
                            tile_position=(32 * h, 0),
                        )
                    else:
                        pb = sub * 64
                        nc.tensor.matmul(
                            sT_ps[:, sub, off:CH],
                            lhsT=kt_sb[pb : pb + HD, pr, ts(i, 128)],
                            rhs=qt_sb[pb : pb + HD, pr, c * CH + off : (c + 1) * CH],
                            start=True,
                            stop=True,
                        )
                ei = work.tile([128, 2, CH], I16, tag="exp", bufs=8, name="e")
                e = ei.bitcast(BF16)
                # the two head-pairs' exps run CONCURRENTLY on different
                # engines: pr0 = ScalarE table exp, pr1 = DVE Schraudolph
                # (bf16 exp bits via fma + f32->i16 round); both ~1.1us, so
                # the k-tile exp pace halves vs one engine doing both.
                dsc = 1.0 / (SQ8 * SK8) if SC_FP8 else 1.0
                if pr == 0:
                    nc.scalar.activation(
                        out=e[:, :, off:CH], in_=sT_ps[:, :, off:CH], func=Exp,
                        scale=dsc,
                    )
                else:
                    nc.vector.tensor_scalar(
                        out=ei[:, :, off:CH],
                        in0=sT_ps[:, :, off:CH],
                        scalar1=A16 * dsc,
                        scalar2=B16,
                        op0=mybir.AluOpType.mult,
                        op1=mybir.AluOpType.add,
                    )
                if 128 * i + 128 > CH * c + off:  # crosses the diagonal: mask
                    # only the 128-wide crossing block needs masking;
                    # columns beyond it are fully unmasked
                    mw = min(128, CH - off)
                    nc.gpsimd.affine_select(
                        out=e[:, :, off : off + mw],
                        in_=e[:, :, off : off + mw],
                        pattern=[[0, 2], [1, mw]],
                        base=0,
                        channel_multiplier=-1,
                        compare_op=mybir.AluOpType.is_ge,
                        fill=0.0,
                    )
                exps[pr][i] = (e, off)

        def ctx(i):
            # 2 heads col-tiled per bank (concurrent on array col halves);
            # start=True clears has_written per written region, so each
            # head's first matmul of the chunk needs it
            for pr in prs:
                e, off = exps[pr][i]
                for sub in range(2):
                    nc.tensor.matmul(
                        ctx2_ps[64 * sub : 64 * sub + 64, pr, off:CH],
                        lhsT=v_sb[:, i, 2 * pr + sub, :],
                        rhs=e[:, sub, off:CH],
                        start=(i == 0),
                        stop=(i == nkt - 1),
                        tile_position=(0, 64 * sub),
                    )
            # denominators: 4-way col-tiled M=1 matmuls, head h at partition 32h.
            # For paired k-tiles (both off=0), DVE pre-sums e(i-1)+e(i) and one
            # den wave covers both; otherwise one wave per k-tile.
            paired = DEN_PAIR and i < 4 * c and i % 2 == 1
            skip = DEN_PAIR and i < 4 * c and i % 2 == 0
            if skip:
                return
            first_den = (i == 0) or (DEN_PAIR and c > 0 and i == 1)
            if paired:
                rhs_by_pr = {}
                for pr in prs:
                    e_a, _ = exps[pr][i - 1]
                    e_b, _ = exps[pr][i]
                    es = work.tile([128, 2, CH], BF16, tag="esum", bufs=2, name="esum")
                    nc.vector.tensor_add(out=es, in0=e_a, in1=e_b)
                    rhs_by_pr[pr] = es
            else:
                rhs_by_pr = {pr: exps[pr][i][0] for pr in prs}
            for pr in prs:
                e, off = exps[pr][i]
                if paired:
                    e, off = rhs_by_pr[pr], 0
                for sub in range(2):
                    h = 2 * pr + sub
                    nc.tensor.matmul(
                        den_ps[32 * h : 32 * h + 1, off:CH],
                        lhsT=wu_sb[:, 0:1],
                        rhs=e[:, sub, off:CH],
                        start=first_den,
                        stop=(i == nkt - 1),
                        tile_position=(0, 32 * h),
                    )

        steps = nkt + PIPE
        for i in range(steps):
            if i < nkt:
                scores(i)
            if i >= PIPE:
                ctx(i - PIPE)
            if fl:
                k = min(2, max(1, -(-len(fl) // (steps - i))))
                for _ in range(k):
                    if fl:
                        fl.popleft()()
        while fl:
            fl.popleft()()

    def tail_norm_parts(c):
        """Denominator eviction + normalize thunks for chunk c."""
        parts = []
        den_sb = small.tile([97, CH], MM_DT, tag="den", name="den_sb")

        def den_evict():
            # denominators live at partitions 0/32/64/96 of den_ps; evict the
            # whole 97-row band in one op (ScalarE: DVE is exp-loaded)
            nc.scalar.copy(out=den_sb, in_=den_ps[0:97, :])

        parts.append(den_evict)
        for pr in range(2):

            def norm(pr=pr, c=c):
                bc_ps = psum.tile([128, CH], F32, tag="sT", bufs=2, name="bc_ps")
                for sub in range(2):
                    p = 32 * (2 * pr + sub)  # weight and fmap must share start partition
                    nc.tensor.matmul(
                        bc_ps[64 * sub : 64 * sub + 64, :],
                        lhsT=wu_sb[p : p + 1, 0:HD],
                        rhs=den_sb[p : p + 1, :],
                        start=True,
                        stop=True,
                        tile_position=(p, 64 * sub),
                    )
                rcp_sb = small.tile([128, CH], F32, tag="rcp", name="rcp_sb")
                nc.vector.reciprocal_approx_fast(out=rcp_sb, in_=bc_ps)
                nc.vector.tensor_mul(
                    out=ctxT_sb[:, pr, ts(c, CH)],
                    in0=ctx2_ps[:, pr, :],
                    in1=rcp_sb,
                )

            parts.append(norm)
        return parts

    def run_all(thunks):
        for th in thunks:
            th()

    # ---- pipeline: projections + out-projections drip into the exp-paced
    # attention stream; only the norm chain sits between phases -----------
    # piece layout: [0:8] = Q/K chain halves, [8:12] = V chains.  A chunk's
    # V tiles are first consumed by its OWN attention (ctx(c, 4c..) at step
    # 4c+PIPE), so V(c) rides at the front of attn(c)'s fillers — this
    # drains the overloaded early phases and feeds the starved attn(3).
    pp1, pp2, pp3 = proj_pieces(1), proj_pieces(2), proj_pieces(3)

    def wu_fill(n):
        def f():
            ps = psum.tile([128, WU_N], F32, tag="aux", bufs=1, name="wf_ps")
            for _ in range(n):
                nc.tensor.matmul(
                    ps, lhsT=wu_sb, rhs=wu_sb[:, 0:WU_N], start=True, stop=True
                )

        return f

    run_all(proj_chains(0))
    emit_attn(0, pp1[0:8])
    # each chunk's norm chain rides as the FIRST fillers of the next chunk's
    # attention: den-evict/bc/recip/mul land in steps 0-2, before ctx(c+1,0)
    # at step PIPE needs the ctx2/den psum banks back -- no boundary stall
    emit_attn(1, tail_norm_parts(0) + pp1[8:12] + pp2[0:8])
    emit_attn(2, tail_norm_parts(1) + pp2[8:12] + oproj_pieces(0) + pp3[0:8])
    # chunk 3 has no next-chunk projections to drip, so it takes both
    # trailing out-projection chunks instead of junk filler matmuls
    emit_attn(3, tail_norm_parts(2) + pp3[8:12] + oproj_pieces(1) + oproj_pieces(2))
    # final norm: den-evict first, filler burst covers the bc/recip/mul
    # latency so the PE stays warm until the tail out-projections' inputs
    parts = tail_norm_parts(3)
    parts[0]()
    tf_ps = psum.tile([128, WU_N], F32, tag="sT", bufs=2, name="tf_ps")
    for r in range(TAILFILL):
        nc.tensor.matmul(
            tf_ps, lhsT=wu_sb, rhs=wu_sb[:, 0:WU_N], start=True, stop=True
        )
    run_all(parts[1:])
    run_all(oproj_pieces(3))

    psum.release()
    small.release()
    work.release()
    acts.release()
    consts.release()


_LDW_PATCHED = False


def _maybe_enable_ldw_opt():
    """Optionally flip walrus's --enable-ldw-opt (fast weight load)."""
    global _LDW_PATCHED
    if _LDW_PATCHED or os.environ.get("BASS_LDW_OPT", "0") != "1":
        return
    import concourse.bass_utils as bu

    orig = bu.run_command

    def run_command(cmd, *a, **kw):
        cmd = [
            "--enable-ldw-opt=true" if c == "--enable-ldw-opt=false" else c
            for c in cmd
        ]
        return orig(cmd, *a, **kw)

    bu.run_command = run_command
    _LDW_PATCHED = True


def build_nc():
    _maybe_enable_ldw_opt()
    nc = bacc.Bacc("TRN2", target_bir_lowering=False, debug=False)
    io = {
        "xtc0": nc.dram_tensor("xtc0", [128, KT, CH], XT_DT, kind="ExternalInput").ap(),
        "xtr": nc.dram_tensor("xtr", [128, KT, S - CH], XT_DT, kind="ExternalInput").ap(),
        "wv": nc.dram_tensor("wv", [128, KT, DG], MM_DT, kind="ExternalInput").ap(),
    }
    if QK_FP8:
        io["wq"] = nc.dram_tensor("wq", [128, KT // 2, 2, 2, 128], FP8, kind="ExternalInput").ap()
        io["wk"] = nc.dram_tensor("wk", [128, KT // 2, 2, 2, 128], FP8, kind="ExternalInput").ap()
        io["xt8c0"] = nc.dram_tensor("xt8c0", [128, KT // 2, 2, CH], FP8, kind="ExternalInput").ap()
        io["xt8r"] = nc.dram_tensor("xt8r", [128, KT // 2, 2, S - CH], FP8, kind="ExternalInput").ap()
    else:
        io["wq"] = nc.dram_tensor("wq", [128, KT, DG], MM_DT, kind="ExternalInput").ap()
        io["wk"] = nc.dram_tensor("wk", [128, KT, DG], MM_DT, kind="ExternalInput").ap()
    io.update({
        "wo": nc.dram_tensor("wo", [128, 2, 1024], MM_DT, kind="ExternalInput").ap(),
        "bq": nc.dram_tensor("bq", [128, 2], F32, kind="ExternalInput").ap(),
        "bk": nc.dram_tensor("bk", [128, 2], F32, kind="ExternalInput").ap(),
        "vb": nc.dram_tensor("vb", [128, HPG, HD], F32, kind="ExternalInput").ap(),
        "out": nc.dram_tensor("out", [S, D], MM_DT, kind="ExternalOutput").ap(),
    })
    with tile.TileContext(nc) as tc, nc.allow_low_precision(
        reason="reduced-precision matmul operand pipeline; accumulation stays fp32"
    ):
        build_kernel_body(nc, tc, io)
    nc.compile()
    return nc


_NC = None


def get_nc():
    global _NC
    if _NC is None:
        _NC = build_nc()
    return _NC


def _tile_rows(a, p=128, dt=None):
    """[R, N] -> [128, R//128, N] with row r = kt*128 + p."""
    r, n = a.shape
    return np.ascontiguousarray(a.reshape(r // p, p, n).transpose(1, 0, 2)).astype(
        dt if dt is not None else _np_dt()
    )


def _xt_dt():
    import ml_dtypes

    return ml_dtypes.float8_e4m3fn if XT_FP8 else _np_dt()


def _pack_w8(W, sc):
    """[1024, 256] -> [128p, 4j, 2t, 2s, 128m] fp8 (DoubleRow pairs of k-tiles)."""
    import ml_dtypes

    return np.ascontiguousarray(
        (W * sc).reshape(4, 2, 128, 2, 128).transpose(2, 0, 3, 1, 4)
    ).astype(ml_dtypes.float8_e4m3fn)


def _pack_x8(xT):
    """[1024, 2048] -> [128p, 4j, 2s, 2048] fp8."""
    import ml_dtypes

    return np.ascontiguousarray(
        xT.reshape(4, 2, 128, S).transpose(2, 0, 1, 3)
    ).astype(ml_dtypes.float8_e4m3fn)


def shard_inputs(x, Wq, bq, Wk, bk, Wv, bv, Wo, bo):
    scale = 1.0 / np.sqrt(np.float32(HD))
    in_maps = []
    for core in range(8):
        b, g = divmod(core, GROUPS)
        sl = slice(g * DG, (g + 1) * DG)
        vb = np.ascontiguousarray(
            np.broadcast_to(bv[sl].reshape(HPG, HD)[None], (128, HPG, HD))
        ).astype(np.float32)
        xT = np.ascontiguousarray(x[b].T)
        if QK_FP8:
            x8 = _pack_x8(xT)
            Wq_s, Wk_s = Wq[:, sl] * scale, np.asarray(Wk[:, sl])
            bq_s, bk_s = bq[sl] * scale, np.asarray(bk[sl])
            if SC_FP8:
                # head-strip column order: slot s holds dims [32s,32s+32)
                # of each head; biases pre-scaled by the fp8 range factor
                perm = np.array(
                    [h * 64 + s_ * 32 + p for s_ in range(2) for h in range(4) for p in range(32)]
                )
                Wq_s, Wk_s = Wq_s[:, perm], Wk_s[:, perm]
                bq_s, bk_s = bq_s[perm] * SQ8, bk_s[perm] * SK8
            qk = {
                "wq": _pack_w8(np.ascontiguousarray(Wq_s), SCQ),
                "wk": _pack_w8(np.ascontiguousarray(Wk_s), SCK),
                "xt8c0": np.ascontiguousarray(x8[:, :, :, 0:CH]),
                "xt8r": np.ascontiguousarray(x8[:, :, :, CH:S]),
            }
        else:
            qk = {
                "wq": _tile_rows(np.ascontiguousarray(Wq[:, sl]) * scale),
                "wk": _tile_rows(np.ascontiguousarray(Wk[:, sl])),
            }
        xt = _tile_rows(xT, dt=_xt_dt())
        in_maps.append(
            {
                **qk,
                "xtc0": np.ascontiguousarray(xt[:, :, 0:CH]),
                "xtr": np.ascontiguousarray(xt[:, :, CH:S]),
                "wv": _tile_rows(np.ascontiguousarray(Wv[:, sl])),
                "wo": _tile_rows(np.ascontiguousarray(Wo[sl, :])),
                "bq": np.ascontiguousarray(
                    (bq_s if QK_FP8 else bq[sl] * scale).reshape(2, 128).T
                ).astype(np.float32),
                "bk": np.ascontiguousarray(
                    (bk_s if QK_FP8 else bk[sl]).reshape(2, 128).T
                ).astype(np.float32),
                "vb": vb,
            }
        )
    return in_maps


LAST_RESULT = None


def kernel(**inputs):
    global LAST_RESULT
    inputs = {k: np.asarray(v) for k, v in inputs.items()}
    nc = get_nc()
    in_maps = shard_inputs(**inputs)
    trace = bool(int(os.environ.get("BASS_KERNEL_TRACE", "0")))
    res = run_bass_kernel_spmd(nc, in_maps, core_ids=list(range(8)), trace=trace)
    LAST_RESULT = res
    parts = [res.results[c]["out"].astype(np.float32) for c in range(8)]
    out = np.stack(
        [
            parts[0] + parts[1] + parts[2] + parts[3],
            parts[4] + parts[5] + parts[6] + parts[7],
        ]
    )
    return (out + inputs["bo"]).astype(np.float32)

